# revision 11
# baseline (speedup 1.0000x reference)
"""GCNConv (gnn_message_passing) Trainium2 kernel, 8 NeuronCores SPMD.

Strategy
--------
out = relu(segment_sum(norm * h[col] -> row)),  h = x @ W,
norm = deg^-1/2[row] * ew * deg^-1/2[col]  (self-loops added, weight 2).

Host packs edges sorted by (dest-block, source-quadrant) into fixed
per-(block,quadrant) tile budgets, so one SPMD program serves all 8 cores —
per-core differences live entirely in the data: int16 gather indices plus
DENSE pre-built selection matrices O[e, r] = norm_e * onehot(relrow_e)
(a dense re-encoding of the edge list, built on host, streamed via HWDGE).

Per-core column ROTATION: core c's x table is np.roll'ed so its own rows
sit first; every core's self-loops then land in quadrant 0, which gets a
larger tile budget (TQs[0]) while quadrants 1-3 stay lean.

Device (per core):
  Phase 1: h = x @ W for ALL nodes (replicated compute on the core's
           rotated table), bf16, stored to 4 DRAM quadrant tables
           (<=25088 rows so dma_gather idx fits int16).
  Phase 2: per group of G blocks: dma_gather 256B h-rows per quadrant
           (SWDGE packed descriptors, <=1024 rows per call, calls rotated
           across 4 SWDGE queues with sem lanes pinned to queues), stream
           the pre-built selection matrices per block, PE matmuls
           accumulate the per-block segment sums in PSUM, ACT ReLU, store.

Output rows are block-contiguous per core; host concatenates and trims.
"""

import numpy as np
import ml_dtypes

BF16 = ml_dtypes.bfloat16

# ---------------------------------------------------------------- config --


class Cfg:
    def __init__(self, N, E, NG, G, D=128, ncores=8, gcall_tiles=7,
                 nqueues=4):
        self.N, self.E, self.D, self.NCORES = N, E, D, ncores
        self.NG, self.G = NG, G          # groups per core, blocks per group
        self.BPC = NG * G                # blocks per core
        self.NB = ncores * self.BPC      # total blocks
        self.NP = self.NB * 128          # padded node count
        assert self.NB % 4 == 0
        self.NQ = self.NP // 4           # quadrant rows
        assert self.NQ <= 32767, "gather idx must fit int16"
        assert self.NP >= N
        assert self.NP % ncores == 0
        self.GCALL = gcall_tiles         # tiles (128 idx) per gather call
        self.NQUEUES = nqueues           # SWDGE queues to rotate over


REAL = Cfg(N=100000, E=1600000, NG=14, G=7)

# ------------------------------------------------------------- host pack --


def _pack(cfg, edge_index, edge_weight):
    """Returns TQs (per-quadrant tile budgets), idx (wrapped int16 gather
    indices) and otab (dense per-tile selection matrices), per core."""
    N, NQ, NB, NP = cfg.N, cfg.NQ, cfg.NB, cfg.NP
    BPC, G, NC = cfg.BPC, cfg.G, cfg.NCORES
    SPC = NP // NC                       # rows per core (rotation step)
    row = np.asarray(edge_index[0], np.int64)
    col = np.asarray(edge_index[1], np.int64)
    wgt = np.asarray(edge_weight, np.float32)
    # degree includes the implicit self-loop of weight 2 (improved=True)
    deg = np.bincount(row, weights=wgt.astype(np.float64), minlength=N) + 2.0
    dinv = (1.0 / np.sqrt(deg)).astype(np.float32)
    norm = dinv[row] * wgt * dinv[col]
    norm_self = dinv * dinv * 2.0        # per-node self-loop message weight

    blk = row >> 7
    core = blk // BPC
    # rotated column index: core c stores x rolled by c*SPC, so its own
    # rows occupy [0, SPC) -> all self loops fall in quadrant 0.
    rcol = (col - core * SPC) % NP
    quad = rcol // NQ

    key = blk * 4 + quad
    cnts = np.bincount(key, minlength=NB * 4)
    cnts4 = cnts.reshape(NB, 4)
    TQs = [int(np.ceil(cnts4[:, q].max() / 128)) for q in range(4)]
    S = sum(TQs)                          # tiles per block
    cum = np.concatenate([[0], np.cumsum(TQs)])
    TILES = BPC * S                       # per core
    SLOTS = TILES * 128
    GS = G * S * 128                      # slots per group

    order = np.argsort(key, kind="stable")
    krt = key[order]
    start = np.zeros(NB * 4, np.int64)
    start[1:] = np.cumsum(cnts)[:-1]
    pos = np.arange(row.size) - start[krt]
    b = krt >> 2
    q = krt & 3
    ocore = b // BPC
    brem = b % BPC
    g = brem // G
    j = brem % G
    TQa = np.array(TQs, np.int64)
    cuma = cum[:-1]
    # gather slot layout per group: [q0 run][q1 run][q2 run][q3 run],
    # each q run j-major (j*TQs[q]*128 slots)
    slot = (g * GS + cuma[q] * G * 128 + j * TQa[q] * 128 + pos)
    lane = pos % 128

    idxf = np.zeros((NC, SLOTS), np.int16)
    idxf[ocore, slot] = (rcol[order] - q * NQ).astype(np.int16)
    # dma_gather wrapped index layout: unwrapped[i] = idx[i%16, i//16],
    # replicated across the 8 gpsimd cores (16 partitions each).
    idxw = idxf.reshape(NC, SLOTS // 16, 16).transpose(0, 2, 1)
    idxw = np.ascontiguousarray(np.tile(idxw, (1, 8, 1)))

    # dense selection matrices, j-major tile order with a leading self-diag
    # tile per block: otau = (g*G + j)*(S+1) + {0: self, 1+cum[q]+t: edges}
    t_in_q = pos // 128
    otau = (g * G + j) * (S + 1) + 1 + cuma[q] + t_in_q
    relrow = (row[order] & 127)
    OT = BPC * (S + 1)
    otab = np.zeros((NC, 128, OT * 128), BF16)
    otab[ocore, lane, otau * 128 + relrow] = norm[order].astype(BF16)
    # self diagonals: node n -> core cn, local block bl, lane ln
    n = np.arange(N, dtype=np.int64)
    cn = (n >> 7) // BPC
    bl = (n >> 7) % BPC
    ln = n & 127
    sot = bl * (S + 1)
    otab[cn, ln, sot * 128 + ln] = norm_self.astype(BF16)
    return TQs, idxw, otab


# -------------------------------------------------------------- bass prog --


def _build(cfg, TQs):
    from concourse import bacc, mybir, tile
    from concourse import tile_sem_assignment as tsa

    # Pin each dma_gather's DMASW sem lane to its SWDGE queue: Tile's
    # round-robin lane rotation runs in *scheduled* order, which need not
    # match trace order, and the SWDGE ucode requires a sem to be fed from
    # a single queue. All Pool-engine DMAs in this program are gathers, so
    # lane == queue_num is safe.
    if not getattr(tsa, "_gcn_lane_patch", False):
        _orig_assign = tsa.TileClockTick._assign_tick

        def _patched_assign(self, inst):
            if isinstance(inst, mybir.InstDMAGatherAnt):
                self.next_sw_dma_idx = inst.queue_num % self.swdge_sem_count
            return _orig_assign(self, inst)

        tsa.TileClockTick._assign_tick = _patched_assign
        tsa._gcn_lane_patch = True

    dt = mybir.dt
    D, NP, NQ = cfg.D, cfg.NP, cfg.NQ
    G, NG = cfg.G, cfg.NG
    S = sum(TQs)
    cum = [0]
    for t in TQs:
        cum.append(cum[-1] + t)
    TILES = cfg.BPC * S
    SLOTS = TILES * 128
    GS = G * S                        # tiles per group
    GCALL = cfg.GCALL
    for q in range(4):
        assert (G * TQs[q]) % GCALL == 0, (TQs, GCALL)
    assert GCALL * 128 <= 1024

    nc = bacc.Bacc("TRN2", target_bir_lowering=False, debug=False,
                   num_devices=cfg.NCORES, num_swdge_queues=cfg.NQUEUES)

    xb = nc.dram_tensor("xb", [NP, D], dt.bfloat16, kind="ExternalInput").ap()
    wb = nc.dram_tensor("wb", [D, D], dt.bfloat16, kind="ExternalInput").ap()
    idx_d = nc.dram_tensor("idx", [128, SLOTS // 16], dt.int16,
                           kind="ExternalInput").ap()
    OT = cfg.BPC * (S + 1)
    o_d = nc.dram_tensor("otab", [128, OT * 128], dt.bfloat16,
                         kind="ExternalInput").ap()
    out_d = nc.dram_tensor("out", [cfg.BPC * 128, D], dt.float32,
                           kind="ExternalOutput").ap()
    hq = [nc.dram_tensor(f"hq{q}", [NQ, D], dt.bfloat16).ap()
          for q in range(4)]

    qctr = [0]

    def next_q():
        qctr[0] += 1
        return qctr[0] % cfg.NQUEUES

    with tile.TileContext(nc) as tc:
        with tc.tile_pool(name="const", bufs=1) as cpool, \
             tc.tile_pool(name="xT", bufs=3) as xpool, \
             tc.tile_pool(name="hs", bufs=3) as hpool, \
             tc.tile_pool(name="psum1", bufs=4, space="PSUM") as p1pool, \
             tc.tile_pool(name="psum2", bufs=4, space="PSUM") as p2pool, \
             tc.tile_pool(name="idxp", bufs=2) as ipool, \
             tc.tile_pool(name="hself", bufs=3) as hpool2, \
             tc.tile_pool(name="slab", bufs=3) as spool, \
             tc.tile_pool(name="otile", bufs=4) as opool, \
             tc.tile_pool(name="outg", bufs=2) as gpool:

            wsb = cpool.tile([D, D], dt.bfloat16, tag="w")
            nc.sync.dma_start(out=wsb[:], in_=wb)

            # ---------------- phase 1: h = x @ W, all nodes, bf16 ---------
            XSTEP = 1024                 # 8 blocks per step
            for s in range(NP // XSTEP):
                r0 = s * XSTEP
                xT = xpool.tile([128, XSTEP], dt.bfloat16, tag="xT")
                nc.sync.dma_start_transpose(out=xT[:], in_=xb[r0:r0 + XSTEP, :])
                hs = hpool.tile([128, XSTEP], dt.bfloat16, tag="hs")
                for half in range(2):
                    ps = p1pool.tile([128, 512], dt.float32, space="PSUM",
                                     tag="ps1")
                    for jj in range(4):
                        nc.tensor.matmul(
                            out=ps[:, jj * 128:(jj + 1) * 128],
                            lhsT=xT[:, (half * 4 + jj) * 128:
                                    (half * 4 + jj + 1) * 128],
                            rhs=wsb[:], start=True, stop=True)
                    dst = hs[:, half * 512:(half + 1) * 512]
                    if half == 0:
                        nc.vector.tensor_copy(out=dst, in_=ps[:])
                    else:
                        nc.scalar.copy(out=dst, in_=ps[:])
                # store to quadrant tables (split at quadrant boundaries)
                seg0 = 0
                while seg0 < XSTEP:
                    q = (r0 + seg0) // NQ
                    qend = min(XSTEP, (q + 1) * NQ - r0)
                    src = hs[:, seg0:qend].rearrange("l (b f) -> l b f", f=D)
                    dst = hq[q][r0 + seg0 - q * NQ: r0 + qend - q * NQ, :] \
                        .rearrange("(b l) f -> l b f", l=128)
                    nc.scalar.dma_start(out=dst, in_=src)
                    seg0 = qend

            # ---------------- phase 2: gather / matmul / relu -------------
            for g in range(NG):
                idxsb = ipool.tile([128, GS * 8], dt.int16, tag="idx")
                nc.sync.dma_start(
                    out=idxsb[:],
                    in_=idx_d[:, g * (GS * 8):(g + 1) * (GS * 8)])
                # contiguous self rows: core's own h lives at rotated
                # rows [0, BPC*128) of hq0 — static addresses on every core
                hself = hpool2.tile([128, G * 128], dt.bfloat16, tag="hself")
                nc.scalar.dma_start(
                    out=hself[:].rearrange("l (j f) -> l j f", f=D),
                    in_=hq[0][g * G * 128:(g + 1) * G * 128, :]
                    .rearrange("(j l) f -> l j f", l=128))
                slabs = []
                for q in range(4):
                    GTq = G * TQs[q]
                    slab = spool.tile([128, GTq * 128], dt.bfloat16,
                                      tag=f"slab{q}")
                    for t0 in range(0, GTq, GCALL):
                        c0 = (cum[q] * G + t0) * 8
                        nc.gpsimd.dma_gather(
                            out_ap=slab[:, t0 * 128:(t0 + GCALL) * 128]
                            .rearrange("p (t e) -> p t e", e=D),
                            in_ap=hq[q],
                            idxs_ap=idxsb[:, c0:c0 + GCALL * 8],
                            num_idxs=GCALL * 128,
                            num_idxs_reg=GCALL * 128,
                            elem_size=D,
                            queue_num=next_q())
                    slabs.append(slab)
                outg = gpool.tile([128, G * 128], dt.float32, tag="outg")
                for j in range(G):
                    oj = opool.tile([128, (S + 1) * 128], dt.bfloat16,
                                    tag="oj")
                    ot0 = ((g * G + j) * (S + 1)) * 128
                    nc.sync.dma_start(out=oj[:],
                                      in_=o_d[:, ot0:ot0 + (S + 1) * 128])
                    ps = p2pool.tile([128, 128], dt.float32, space="PSUM",
                                     tag="ps2")
                    nc.tensor.matmul(
                        out=ps[:], lhsT=oj[:, 0:128],
                        rhs=hself[:, j * 128:(j + 1) * 128],
                        start=True, stop=False)
                    for q in range(4):
                        for t in range(TQs[q]):
                            nc.tensor.matmul(
                                out=ps[:],
                                lhsT=oj[:, (1 + cum[q] + t) * 128:
                                        (2 + cum[q] + t) * 128],
                                rhs=slabs[q][:, (j * TQs[q] + t) * 128:
                                             (j * TQs[q] + t + 1) * 128],
                                start=False,
                                stop=(q == 3 and t == TQs[3] - 1))
                    nc.scalar.activation(
                        out=outg[:, j * 128:(j + 1) * 128], in_=ps[:],
                        func=mybir.ActivationFunctionType.Relu)
                nc.sync.dma_start(
                    out=out_d[g * G * 128:(g + 1) * G * 128, :]
                    .rearrange("(j l) f -> l j f", l=128),
                    in_=outg[:].rearrange("l (j f) -> l j f", f=D))

    nc.compile()
    return nc


# ------------------------------------------------------------------ run --


def _prepare_inputs(cfg, x, edge_index, edge_weight, weight):
    TQs, idxw, otab = _pack(cfg, edge_index, edge_weight)
    SPC = cfg.NP // cfg.NCORES
    xbase = np.zeros((cfg.NP, cfg.D), np.float32)
    xbase[:cfg.N] = np.asarray(x, np.float32)
    wb = np.asarray(weight).astype(BF16)
    in_maps = []
    for c in range(cfg.NCORES):
        xc = np.roll(xbase, -c * SPC, axis=0).astype(BF16)
        in_maps.append({
            "xb": np.ascontiguousarray(xc), "wb": wb,
            "idx": np.ascontiguousarray(idxw[c]),
            "otab": np.ascontiguousarray(otab[c]),
        })
    return TQs, in_maps


_CACHE = {}


def _get_program(cfg, TQs):
    key = (cfg.N, cfg.NG, cfg.G, tuple(TQs), cfg.GCALL, cfg.NQUEUES)
    if key not in _CACHE:
        _CACHE[key] = _build(cfg, TQs)
    return _CACHE[key]


def run(cfg, x, edge_index, edge_weight, weight, trace=False, **kw):
    from concourse.bass_utils import run_bass_kernel_spmd
    TQs, in_maps = _prepare_inputs(cfg, x, edge_index, edge_weight, weight)
    nc = _get_program(cfg, TQs)
    res = run_bass_kernel_spmd(nc, in_maps, list(range(cfg.NCORES)),
                               trace=trace, **kw)
    out = np.concatenate([res.results[c]["out"] for c in range(cfg.NCORES)],
                         axis=0)[:cfg.N]
    return out.astype(np.float32), res


def kernel(x, edge_index, edge_weight, weight):
    out, _ = run(REAL, x, edge_index, edge_weight, weight)
    return out


# revision 12
# speedup vs baseline: 1.4342x; 1.4342x over previous
"""GCNConv (gnn_message_passing) Trainium2 kernel, 8 NeuronCores SPMD.

Strategy
--------
out = relu(segment_sum(norm * h[col] -> row)),  h = x @ W,
norm = deg^-1/2[row] * ew * deg^-1/2[col]  (self-loops added, weight 2).

Host packs edges sorted by (dest-block, source-quadrant) into fixed
per-(block,quadrant) tile budgets, so one SPMD program serves all 8 cores —
per-core differences live entirely in the data: int16 gather indices plus
DENSE pre-built selection matrices O[e, r] = norm_e * onehot(relrow_e)
(a dense re-encoding of the edge list, built on host, streamed via HWDGE).

Per-core column ROTATION: core c's x table is np.roll'ed so its own rows
sit first; every core's self-loops then land in quadrant 0, which gets a
larger tile budget (TQs[0]) while quadrants 1-3 stay lean.

Device (per core):
  Phase 1: h = x @ W for ALL nodes (replicated compute on the core's
           rotated table), bf16, stored to 4 DRAM quadrant tables
           (<=25088 rows so dma_gather idx fits int16).
  Phase 2: per group of G blocks: dma_gather 256B h-rows per quadrant
           (SWDGE packed descriptors, <=1024 rows per call, calls rotated
           across 4 SWDGE queues with sem lanes pinned to queues), stream
           the pre-built selection matrices per block, PE matmuls
           accumulate the per-block segment sums in PSUM, ACT ReLU, store.

Output rows are block-contiguous per core; host concatenates and trims.
"""

import numpy as np
import ml_dtypes

BF16 = ml_dtypes.bfloat16

# ---------------------------------------------------------------- config --


class Cfg:
    def __init__(self, N, E, NG, G, D=128, ncores=8, gcall_tiles=7,
                 nqueues=4):
        self.N, self.E, self.D, self.NCORES = N, E, D, ncores
        self.NG, self.G = NG, G          # groups per core, blocks per group
        self.BPC = NG * G                # blocks per core
        self.NB = ncores * self.BPC      # total blocks
        self.NP = self.NB * 128          # padded node count
        assert self.NB % 4 == 0
        self.NQ = self.NP // 4           # quadrant rows
        assert self.NQ <= 32767, "gather idx must fit int16"
        assert self.NP >= N
        assert self.NP % ncores == 0
        self.GCALL = gcall_tiles         # tiles (128 idx) per gather call
        self.NQUEUES = nqueues           # SWDGE queues to rotate over


REAL = Cfg(N=100000, E=1600000, NG=14, G=7)

# ------------------------------------------------------------- host pack --


def _pack(cfg, edge_index, edge_weight):
    """Returns TQs (per-quadrant tile budgets), idx (wrapped int16 gather
    indices) and otab (dense per-tile selection matrices), per core."""
    N, NQ, NB, NP = cfg.N, cfg.NQ, cfg.NB, cfg.NP
    BPC, G, NC = cfg.BPC, cfg.G, cfg.NCORES
    SPC = NP // NC                       # rows per core (rotation step)
    row = np.asarray(edge_index[0], np.int64)
    col = np.asarray(edge_index[1], np.int64)
    wgt = np.asarray(edge_weight, np.float32)
    # degree includes the implicit self-loop of weight 2 (improved=True)
    deg = np.bincount(row, weights=wgt.astype(np.float64), minlength=N) + 2.0
    dinv = (1.0 / np.sqrt(deg)).astype(np.float32)
    norm = dinv[row] * wgt * dinv[col]
    norm_self = dinv * dinv * 2.0        # per-node self-loop message weight

    blk = row >> 7
    core = blk // BPC
    # rotated column index: core c stores x rolled by c*SPC, so its own
    # rows occupy [0, SPC) -> all self loops fall in quadrant 0.
    rcol = (col - core * SPC) % NP
    quad = rcol // NQ

    key = blk * 4 + quad
    cnts = np.bincount(key, minlength=NB * 4)
    cnts4 = cnts.reshape(NB, 4)
    TQs = [int(np.ceil(cnts4[:, q].max() / 128)) for q in range(4)]
    S = sum(TQs)                          # tiles per block
    cum = np.concatenate([[0], np.cumsum(TQs)])
    TILES = BPC * S                       # per core
    SLOTS = TILES * 128
    GS = G * S * 128                      # slots per group

    order = np.argsort(key, kind="stable")
    krt = key[order]
    start = np.zeros(NB * 4, np.int64)
    start[1:] = np.cumsum(cnts)[:-1]
    pos = np.arange(row.size) - start[krt]
    b = krt >> 2
    q = krt & 3
    ocore = b // BPC
    brem = b % BPC
    g = brem // G
    j = brem % G
    TQa = np.array(TQs, np.int64)
    cuma = cum[:-1]
    # gather slot layout per group: [q0 run][q1 run][q2 run][q3 run],
    # each q run j-major (j*TQs[q]*128 slots)
    slot = (g * GS + cuma[q] * G * 128 + j * TQa[q] * 128 + pos)
    lane = pos % 128

    idxf = np.zeros((NC, SLOTS), np.int16)
    idxf[ocore, slot] = (rcol[order] - q * NQ).astype(np.int16)
    # dma_gather wrapped index layout: unwrapped[i] = idx[i%16, i//16],
    # replicated across the 8 gpsimd cores (16 partitions each).
    idxw = idxf.reshape(NC, SLOTS // 16, 16).transpose(0, 2, 1)
    idxw = np.ascontiguousarray(np.tile(idxw, (1, 8, 1)))

    # dense selection matrices, j-major tile order with a leading self-diag
    # tile per block: otau = (g*G + j)*(S+1) + {0: self, 1+cum[q]+t: edges}
    t_in_q = pos // 128
    otau = (g * G + j) * (S + 1) + 1 + cuma[q] + t_in_q
    relrow = (row[order] & 127)
    OT = BPC * (S + 1)
    otab = np.zeros((NC, 128, OT * 128), BF16)
    otab[ocore, lane, otau * 128 + relrow] = norm[order].astype(BF16)
    # self diagonals: node n -> core cn, local block bl, lane ln
    n = np.arange(N, dtype=np.int64)
    cn = (n >> 7) // BPC
    bl = (n >> 7) % BPC
    ln = n & 127
    sot = bl * (S + 1)
    otab[cn, ln, sot * 128 + ln] = norm_self.astype(BF16)
    return TQs, idxw, otab


# -------------------------------------------------------------- bass prog --


def _build(cfg, TQs):
    from concourse import bacc, mybir, tile
    from concourse import tile_sem_assignment as tsa

    # Pin each dma_gather's DMASW sem lane to its SWDGE queue: Tile's
    # round-robin lane rotation runs in *scheduled* order, which need not
    # match trace order, and the SWDGE ucode requires a sem to be fed from
    # a single queue. All Pool-engine DMAs in this program are gathers, so
    # lane == queue_num is safe.
    if not getattr(tsa, "_gcn_lane_patch", False):
        _orig_assign = tsa.TileClockTick._assign_tick

        def _patched_assign(self, inst):
            if isinstance(inst, mybir.InstDMAGatherAnt):
                self.next_sw_dma_idx = inst.queue_num % self.swdge_sem_count
            return _orig_assign(self, inst)

        tsa.TileClockTick._assign_tick = _patched_assign
        tsa._gcn_lane_patch = True

    dt = mybir.dt
    D, NP, NQ = cfg.D, cfg.NP, cfg.NQ
    G, NG = cfg.G, cfg.NG
    S = sum(TQs)
    cum = [0]
    for t in TQs:
        cum.append(cum[-1] + t)
    TILES = cfg.BPC * S
    SLOTS = TILES * 128
    GS = G * S                        # tiles per group
    GCALL = cfg.GCALL
    for q in range(4):
        assert (G * TQs[q]) % GCALL == 0, (TQs, GCALL)
    assert GCALL * 128 <= 1024

    nc = bacc.Bacc("TRN2", target_bir_lowering=False, debug=False,
                   num_devices=cfg.NCORES, num_swdge_queues=cfg.NQUEUES)

    xt_d = nc.dram_tensor("xt", [D, NP], dt.bfloat16,
                          kind="ExternalInput").ap()
    wb = nc.dram_tensor("wb", [D, D], dt.bfloat16, kind="ExternalInput").ap()
    idx_d = nc.dram_tensor("idx", [128, SLOTS // 16], dt.int16,
                           kind="ExternalInput").ap()
    OT = cfg.BPC * (S + 1)
    o_d = nc.dram_tensor("otab", [128, OT * 128], dt.bfloat16,
                         kind="ExternalInput").ap()
    out_d = nc.dram_tensor("out", [cfg.BPC * 128, D], dt.float32,
                           kind="ExternalOutput").ap()
    hq = [nc.dram_tensor(f"hq{q}", [NQ, D], dt.bfloat16).ap()
          for q in range(4)]

    qctr = [0]

    def next_q():
        qctr[0] += 1
        return qctr[0] % cfg.NQUEUES

    with tile.TileContext(nc) as tc:
        with tc.tile_pool(name="const", bufs=1) as cpool, \
             tc.tile_pool(name="xT", bufs=3) as xpool, \
             tc.tile_pool(name="hs", bufs=3) as hpool, \
             tc.tile_pool(name="psum1", bufs=4, space="PSUM") as p1pool, \
             tc.tile_pool(name="psum2", bufs=4, space="PSUM") as p2pool, \
             tc.tile_pool(name="idxp", bufs=2) as ipool, \
             tc.tile_pool(name="hself", bufs=3) as hpool2, \
             tc.tile_pool(name="slab", bufs=3) as spool, \
             tc.tile_pool(name="otile", bufs=4) as opool, \
             tc.tile_pool(name="outg", bufs=2) as gpool:

            wsb = cpool.tile([D, D], dt.bfloat16, tag="w")
            nc.sync.dma_start(out=wsb[:], in_=wb)

            # ---------------- phase 1: h = x @ W, all nodes, bf16 ---------
            XSTEP = 1024                 # 8 blocks per step
            for s in range(NP // XSTEP):
                r0 = s * XSTEP
                xT = xpool.tile([128, XSTEP], dt.bfloat16, tag="xT")
                nc.sync.dma_start(out=xT[:], in_=xt_d[:, r0:r0 + XSTEP])
                hs = hpool.tile([128, XSTEP], dt.bfloat16, tag="hs")
                for half in range(2):
                    ps = p1pool.tile([128, 512], dt.float32, space="PSUM",
                                     tag="ps1")
                    for jj in range(4):
                        nc.tensor.matmul(
                            out=ps[:, jj * 128:(jj + 1) * 128],
                            lhsT=xT[:, (half * 4 + jj) * 128:
                                    (half * 4 + jj + 1) * 128],
                            rhs=wsb[:], start=True, stop=True)
                    dst = hs[:, half * 512:(half + 1) * 512]
                    if half == 0:
                        nc.vector.tensor_copy(out=dst, in_=ps[:])
                    else:
                        nc.scalar.copy(out=dst, in_=ps[:])
                # store to quadrant tables (split at quadrant boundaries)
                seg0 = 0
                while seg0 < XSTEP:
                    q = (r0 + seg0) // NQ
                    qend = min(XSTEP, (q + 1) * NQ - r0)
                    src = hs[:, seg0:qend].rearrange("l (b f) -> l b f", f=D)
                    dst = hq[q][r0 + seg0 - q * NQ: r0 + qend - q * NQ, :] \
                        .rearrange("(b l) f -> l b f", l=128)
                    nc.scalar.dma_start(out=dst, in_=src)
                    seg0 = qend

            # ---------------- phase 2: gather / matmul / relu -------------
            for g in range(NG):
                idxsb = ipool.tile([128, GS * 8], dt.int16, tag="idx")
                nc.sync.dma_start(
                    out=idxsb[:],
                    in_=idx_d[:, g * (GS * 8):(g + 1) * (GS * 8)])
                # contiguous self rows: core's own h lives at rotated
                # rows [0, BPC*128) of hq0 — static addresses on every core
                hself = hpool2.tile([128, G * 128], dt.bfloat16, tag="hself")
                nc.scalar.dma_start(
                    out=hself[:].rearrange("l (j f) -> l j f", f=D),
                    in_=hq[0][g * G * 128:(g + 1) * G * 128, :]
                    .rearrange("(j l) f -> l j f", l=128))
                slabs = []
                for q in range(4):
                    GTq = G * TQs[q]
                    slab = spool.tile([128, GTq * 128], dt.bfloat16,
                                      tag=f"slab{q}")
                    for t0 in range(0, GTq, GCALL):
                        c0 = (cum[q] * G + t0) * 8
                        nc.gpsimd.dma_gather(
                            out_ap=slab[:, t0 * 128:(t0 + GCALL) * 128]
                            .rearrange("p (t e) -> p t e", e=D),
                            in_ap=hq[q],
                            idxs_ap=idxsb[:, c0:c0 + GCALL * 8],
                            num_idxs=GCALL * 128,
                            num_idxs_reg=GCALL * 128,
                            elem_size=D,
                            queue_num=next_q())
                    slabs.append(slab)
                outg = gpool.tile([128, G * 128], dt.float32, tag="outg")
                for j in range(G):
                    oj = opool.tile([128, (S + 1) * 128], dt.bfloat16,
                                    tag="oj")
                    ot0 = ((g * G + j) * (S + 1)) * 128
                    eng = nc.sync if j % 2 == 0 else nc.scalar
                    eng.dma_start(out=oj[:],
                                  in_=o_d[:, ot0:ot0 + (S + 1) * 128])
                    ps = p2pool.tile([128, 128], dt.float32, space="PSUM",
                                     tag="ps2")
                    nc.tensor.matmul(
                        out=ps[:], lhsT=oj[:, 0:128],
                        rhs=hself[:, j * 128:(j + 1) * 128],
                        start=True, stop=False)
                    for q in range(4):
                        for t in range(TQs[q]):
                            nc.tensor.matmul(
                                out=ps[:],
                                lhsT=oj[:, (1 + cum[q] + t) * 128:
                                        (2 + cum[q] + t) * 128],
                                rhs=slabs[q][:, (j * TQs[q] + t) * 128:
                                             (j * TQs[q] + t + 1) * 128],
                                start=False,
                                stop=(q == 3 and t == TQs[3] - 1))
                    nc.scalar.activation(
                        out=outg[:, j * 128:(j + 1) * 128], in_=ps[:],
                        func=mybir.ActivationFunctionType.Relu)
                nc.sync.dma_start(
                    out=out_d[g * G * 128:(g + 1) * G * 128, :]
                    .rearrange("(j l) f -> l j f", l=128),
                    in_=outg[:].rearrange("l (j f) -> l j f", f=D))

    nc.compile()
    return nc


# ------------------------------------------------------------------ run --


def _prepare_inputs(cfg, x, edge_index, edge_weight, weight):
    TQs, idxw, otab = _pack(cfg, edge_index, edge_weight)
    SPC = cfg.NP // cfg.NCORES
    xbase = np.zeros((cfg.NP, cfg.D), np.float32)
    xbase[:cfg.N] = np.asarray(x, np.float32)
    wb = np.asarray(weight).astype(BF16)
    in_maps = []
    for c in range(cfg.NCORES):
        xc = np.roll(xbase, -c * SPC, axis=0).astype(BF16)
        in_maps.append({
            "xt": np.ascontiguousarray(xc.T), "wb": wb,
            "idx": np.ascontiguousarray(idxw[c]),
            "otab": np.ascontiguousarray(otab[c]),
        })
    return TQs, in_maps


_CACHE = {}


def _get_program(cfg, TQs):
    key = (cfg.N, cfg.NG, cfg.G, tuple(TQs), cfg.GCALL, cfg.NQUEUES)
    if key not in _CACHE:
        _CACHE[key] = _build(cfg, TQs)
    return _CACHE[key]


def run(cfg, x, edge_index, edge_weight, weight, trace=False, **kw):
    from concourse.bass_utils import run_bass_kernel_spmd
    TQs, in_maps = _prepare_inputs(cfg, x, edge_index, edge_weight, weight)
    nc = _get_program(cfg, TQs)
    res = run_bass_kernel_spmd(nc, in_maps, list(range(cfg.NCORES)),
                               trace=trace, **kw)
    out = np.concatenate([res.results[c]["out"] for c in range(cfg.NCORES)],
                         axis=0)[:cfg.N]
    return out.astype(np.float32), res


def kernel(x, edge_index, edge_weight, weight):
    out, _ = run(REAL, x, edge_index, edge_weight, weight)
    return out
